# revision 1
# baseline (speedup 1.0000x reference)
"""Multi-head attention kernel for Trainium2, batch-parallel across 8 NeuronCores.

Reference (per batch element b, one core each):
  qk = x @ W_qk.T ; q,k = split(qk) ; v = x @ W_v.T
  q,k,v -> [h, n, d] ; q += pos_h ; k += pos_h
  S = q @ k.T * DIM**-0.5 ; mask = outer(m, m) ; masked -> -inf
  P = softmax(S) ; O = P @ v ; out = merge_heads(O) @ W_out.T + b_out

Device strategy (per core):
  - everything in fp32r (TF32-like, full PE rate at free-dim >= 256)
  - x, pos, W_qk, W_v, W_out transposed on-chip via PE-transpose
  - scores computed TRANSPOSED: ST[j, i] = k_h @ q_h.T so the scale and the
    column mask fold into one ACT exp (bias = per-partition mask bias) and
    no vector reductions are needed: softmax row sums come from an appended
    ones-column in the PV matmul (V_aug = [V_h | 1], M=65).
  - normalization (1/s), row mask, and masked-row blend with mean(V) are
    applied per head on the PV output with 2 DVE ops.
  - out-projection accumulates over head dims in PSUM and adds the bias via
    a broadcast tensor during the PSUM->SBUF copy.
"""
import sys

sys.path.insert(0, "/opt/trn_rl_repo")

import numpy as np
from contextlib import ExitStack

B, N, DIM, H = 8, 1024, 1024, 16
D = DIM // H          # 64
E = D + 1             # V_aug block (64 cols of V + ones column)
P = 128
NT = N // P           # 8 n-tiles
KT = DIM // P         # 8 k-tiles
NPAIR = H // 2        # 8 head pairs
SCALE = DIM ** (-0.5)
MB = 30.0             # mask bias magnitude: bias_j = 30*m - 30 in {0, -30}

_NC = None


def _build():
    import concourse.bacc as bacc
    import concourse.bass as bass
    import concourse.mybir as mybir
    import concourse.tile as tile
    from concourse.masks import make_identity

    f32 = mybir.dt.float32
    f32r = mybir.dt.float32r
    bf16 = mybir.dt.bfloat16
    AF = mybir.ActivationFunctionType
    OP = mybir.AluOpType
    ts = bass.ts

    nc = bacc.Bacc()
    x_d = nc.declare_dram_parameter("x", [N, DIM], f32, isOutput=False)
    pos_d = nc.declare_dram_parameter("pos", [N, DIM], f32, isOutput=False)
    maskf_d = nc.declare_dram_parameter("maskf", [N], f32, isOutput=False)
    wqk_d = nc.declare_dram_parameter("W_qk", [2 * DIM, DIM], f32, isOutput=False)
    wv_d = nc.declare_dram_parameter("W_v", [DIM, DIM], f32, isOutput=False)
    wout_d = nc.declare_dram_parameter("W_out", [DIM, DIM], f32, isOutput=False)
    b_d = nc.declare_dram_parameter("b_out", [DIM], f32, isOutput=False)
    out_d = nc.declare_dram_parameter("out", [N, DIM], f32, isOutput=True)

    with ExitStack() as ctx:
        tc = ctx.enter_context(tile.TileContext(nc))
        sing = ctx.enter_context(tc.tile_pool(name="sing", bufs=1))
        rowload = ctx.enter_context(tc.tile_pool(name="rowload", bufs=4))
        pair2 = ctx.enter_context(tc.tile_pool(name="pair2", bufs=3))
        work1 = ctx.enter_context(tc.tile_pool(name="work1", bufs=1))
        work2 = ctx.enter_context(tc.tile_pool(name="work2", bufs=2))
        expp = ctx.enter_context(tc.tile_pool(name="expp", bufs=6))
        ps_mm = ctx.enter_context(tc.tile_pool(name="ps_mm", bufs=2, space="PSUM"))
        ps_st = ctx.enter_context(tc.tile_pool(name="ps_st", bufs=3, space="PSUM"))
        ps_oa = ctx.enter_context(tc.tile_pool(name="ps_oa", bufs=3, space="PSUM"))

        # ---------- constants / small prep ----------
        ident = sing.tile([P, P], f32, tag="ident")
        make_identity(nc, ident)
        ident_bf = sing.tile([P, P], bf16, tag="ident_bf")
        make_identity(nc, ident_bf)

        # maskf in [p, c] layout (j = c*128 + p): exp bias column per j-tile
        mstage = sing.tile([P, NT], f32, tag="mstage")
        nc.sync.dma_start(out=mstage, in_=maskf_d.rearrange("(c p) -> p c", c=NT))
        bias_j = sing.tile([P, NT], f32, tag="bias_j")
        nc.scalar.activation(bias_j, mstage, AF.Copy, bias=-MB, scale=MB)
        # mask rows
        m_row = sing.tile([1, N], f32, tag="m_row")
        nc.sync.dma_start(out=m_row, in_=maskf_d[:])
        omm_row = sing.tile([1, N], f32, tag="omm_row")
        nc.vector.tensor_scalar(omm_row, m_row, -1.0, 1.0, OP.mult, OP.add)
        ommb = sing.tile([P, N], f32, tag="ommb")
        nc.gpsimd.partition_broadcast(ommb, omm_row)
        # mask in the s-collect layout: m_coll[p, i*4+c] = maskf[i*512 + p*4 + c]
        m_coll = sing.tile([P, 8], f32, tag="m_coll")
        nc.sync.dma_start(
            out=m_coll.rearrange("p (i c) -> p i c", i=2),
            in_=maskf_d.rearrange("(i p c) -> p i c", i=2, c=4),
        )
        # bias row -> broadcast over partitions
        b_row = sing.tile([1, DIM], f32, tag="b_row")
        nc.sync.dma_start(out=b_row, in_=b_d[:])
        b_bcast = sing.tile([P, DIM], f32, tag="b_bcast")
        nc.gpsimd.partition_broadcast(b_bcast, b_row)

        # ---------- x^T ----------
        xT = [sing.tile([P, N], bf16, tag=f"xT{kc}", name=f"xT{kc}") for kc in range(KT)]
        for nt in range(NT):
            xr = rowload.tile([P, DIM], f32, tag="rowload")
            nc.sync.dma_start(out=xr, in_=x_d[ts(nt, P), :])
            xb = rowload.tile([P, DIM], bf16, tag="rowload_bf", bufs=2)
            nc.vector.tensor_copy(xb, xr)
            for kc in range(KT):
                tp = ps_mm.tile([P, P], bf16, tag="mm", name="tp")
                nc.tensor.transpose(tp, xb[:, ts(kc, P)], ident_bf)
                nc.vector.tensor_copy(xT[kc][:, ts(nt, P)], tp)

        # ---------- V = x @ W_v.T  (stored as [V_h | 1] x 16 heads) ----------
        V_sb = [sing.tile([P, H * E], bf16, tag=f"V{nt}", name=f"V{nt}") for nt in range(NT)]
        const1 = sing.tile([P, H], f32, tag="const1")
        nc.vector.memset(const1, 1.0)
        for nt in range(NT):
            ones_ap = V_sb[nt].rearrange("p (h e) -> p h e", e=E)[:, :, D:E]
            nc.vector.tensor_copy(ones_ap.squeeze(), const1)
        whalf = [None] * KT
        for dvh in range(2):
            for kc in range(KT):
                whalf[kc] = work1.tile([P, 512], bf16, tag=f"whalf{kc}", name=f"whalf{kc}")
            for rt in range(4):
                wr = rowload.tile([P, DIM], f32, tag="rowload")
                nc.sync.dma_start(
                    out=wr,
                    in_=wv_d[dvh * 512 + rt * P: dvh * 512 + (rt + 1) * P, :])
                wb = rowload.tile([P, DIM], bf16, tag="rowload_bf", bufs=2)
                nc.vector.tensor_copy(wb, wr)
                for kc in range(KT):
                    tp = ps_mm.tile([P, P], bf16, tag="mm", name="tp")
                    nc.tensor.transpose(tp, wb[:, ts(kc, P)], ident_bf)
                    nc.vector.tensor_copy(whalf[kc][:, ts(rt, P)], tp)
            for nt in range(NT):
                acc = ps_mm.tile([P, 512], f32, tag="mm")
                for kc in range(KT):
                    nc.tensor.matmul(acc, xT[kc][:, ts(nt, P)], whalf[kc],
                                     start=(kc == 0), stop=(kc == KT - 1))
                dst = V_sb[nt][:, dvh * 8 * E:].rearrange(
                    "p (h e) -> p h e", e=E)[:, 0:8, 0:D]
                nc.vector.tensor_copy(dst, acc.rearrange("p (h e) -> p h e", e=D))

        # ---------- W_out^T (built early so PE gaps can absorb it) ----------
        woutT = [sing.tile([P, DIM], bf16, tag=f"woT{kc}", name=f"woT{kc}")
                 for kc in range(KT)]
        for rt in range(NT):
            wr = rowload.tile([P, DIM], f32, tag="rowload")
            nc.sync.dma_start(out=wr, in_=wout_d[ts(rt, P), :])
            wb = rowload.tile([P, DIM], bf16, tag="rowload_bf", bufs=2)
            nc.vector.tensor_copy(wb, wr)
            for kc in range(KT):
                tp = ps_mm.tile([P, P], bf16, tag="mm", name="tp")
                nc.tensor.transpose(tp, wb[:, ts(kc, P)], ident_bf)
                nc.vector.tensor_copy(woutT[kc][:, ts(rt, P)], tp)

        # ---------- mean over sequence of V_aug ----------
        ones_col = sing.tile([P, 1], bf16, tag="ones_col")
        constN = sing.tile([P, 1], f32, tag="constN")
        nc.vector.memset(constN, 1.0 / N)
        nc.vector.tensor_copy(ones_col, constN)
        mean_sb = sing.tile([1, H * E], f32, tag="mean_sb")
        for c0, cs in ((0, 512), (512, 512), (1024, H * E - 1024)):
            mp = ps_mm.tile([P, 512], f32, tag="mm")
            for nt in range(NT):
                nc.tensor.matmul(mp[0:1, 0:cs], ones_col, V_sb[nt][:, c0:c0 + cs],
                                 start=(nt == 0), stop=(nt == NT - 1))
            nc.vector.tensor_copy(mean_sb[:, c0:c0 + cs], mp[0:1, 0:cs])
        # per-head mean as a per-partition scalar column [64, H]
        mean_cols = sing.tile([D, H], f32, tag="mean_cols")
        for h in range(H):
            nc.sync.dma_start(out=mean_cols[:, h:h + 1],
                              in_=mean_sb[0:1, h * E:h * E + D])

        # ---------- per head-pair: projections + attention ----------
        otfull = [sing.tile([P, N], bf16, tag=f"otf{kc}", name=f"otf{kc}")
                  for kc in range(KT)]

        for t in range(NPAIR):
            # --- pos^T for this dim-slice ---
            posT = pair2.tile([P, N], f32, tag="posT")
            for nt in range(NT):
                pr = rowload.tile([P, P], f32, tag="posload")
                nc.sync.dma_start(out=pr, in_=pos_d[ts(nt, P), ts(t, P)])
                tp = ps_mm.tile([P, P], f32, tag="mm", name="tp")
                nc.tensor.transpose(tp, pr, ident)
                nc.vector.tensor_copy(posT[:, ts(nt, P)], tp)
            # --- q^T / k^T for this pair (heads 2t, 2t+1) ---
            qT = pair2.tile([P, N], bf16, tag="qT")
            kT = pair2.tile([P, N], bf16, tag="kT")
            for which, wt in ((0, qT), (1, kT)):
                wqr = rowload.tile([P, DIM], f32, tag="rowload")
                nc.sync.dma_start(
                    out=wqr,
                    in_=wqk_d[which * DIM + t * P: which * DIM + (t + 1) * P, :])
                wqb = rowload.tile([P, DIM], bf16, tag="rowload_bf", bufs=2)
                nc.vector.tensor_copy(wqb, wqr)
                wtr = work2.tile([P, DIM], bf16, tag=f"wqkT{which}")
                for kc in range(KT):
                    tp = ps_mm.tile([P, P], bf16, tag="mm", name="tp")
                    nc.tensor.transpose(tp, wqb[:, ts(kc, P)], ident_bf)
                    nc.vector.tensor_copy(wtr[:, ts(kc, P)], tp)
                for half in range(2):
                    acc = ps_mm.tile([P, 512], f32, tag="mm")
                    for kc in range(KT):
                        nc.tensor.matmul(acc, wtr[:, ts(kc, P)],
                                         xT[kc][:, ts(half, 512)],
                                         start=(kc == 0), stop=(kc == KT - 1))
                    nc.vector.tensor_add(wt[:, ts(half, 512)], acc,
                                         posT[:, ts(half, 512)])

            # --- attention, one head at a time ---
            for hs in range(2):
                h = 2 * t + hs
                hoff = hs * D
                oa = [ps_oa.tile([P, 512], f32, tag="oa", name="oa") for _ in range(2)]
                for jt in range(NT):
                    ex = expp.tile([P, N], bf16, tag="ex")
                    va = V_sb[jt][:, h * E:(h + 1) * E]
                    for ih in range(2):
                        st = ps_st.tile([P, 512], f32, tag="st")
                        nc.tensor.matmul(st,
                                         kT[hoff:hoff + D, ts(jt, P)],
                                         qT[hoff:hoff + D, ts(ih, 512)],
                                         start=True, stop=True)
                        nc.scalar.activation(ex[:, ts(ih, 512)], st, AF.Exp,
                                             bias=bias_j[:, jt:jt + 1],
                                             scale=SCALE)
                        nc.tensor.matmul(oa[ih][0:E, :], va, ex[:, ts(ih, 512)],
                                         start=(jt == 0), stop=(jt == NT - 1))
                # s rows: psum row 64 -> sbuf -> [p, c] collect layout
                s_stage = work1.tile([D + 1, N], f32, tag="s_stage")
                s_coll = work2.tile([P, 8], f32, tag="s_coll")
                for ih in range(2):
                    nc.vector.tensor_copy(s_stage[D:D + 1, ts(ih, 512)],
                                          oa[ih][D:D + 1, :])
                    # s_coll[p, i*4+c] = s[i*512 + p*4 + c]
                    nc.sync.dma_start(out=s_coll[:, ih * 4:(ih + 1) * 4],
                                      in_=s_stage[D:D + 1, ts(ih, 512)])
                r_coll = work2.tile([P, 8], f32, tag="r_coll")
                nc.vector.reciprocal(r_coll, s_coll)
                nc.vector.tensor_mul(r_coll, r_coll, m_coll)
                rm_row = work2.tile([1, N], f32, tag="rm_row")
                for ih in range(2):
                    nc.sync.dma_start(
                        out=rm_row[:, ts(ih, 512)].rearrange(
                            "o (p c) -> o p c", p=P, c=4),
                        in_=r_coll[:, ih * 4:(ih + 1) * 4],
                    )
                rmb = work2.tile([D, N], f32, tag="rmb")
                nc.gpsimd.partition_broadcast(rmb, rm_row)
                hscr = work2.tile([D, N], bf16, tag="hscr")
                for ih in range(2):
                    t1 = work2.tile([D, 512], f32, tag="t1")
                    nc.vector.tensor_mul(t1, oa[ih][0:D, :], rmb[:, ts(ih, 512)])
                    nc.vector.scalar_tensor_tensor(
                        hscr[:, ts(ih, 512)], ommb[0:D, ts(ih, 512)],
                        mean_cols[:, h:h + 1], t1, OP.mult, OP.add)
                nc.sync.dma_start(out=otfull[t][hoff:hoff + D, :], in_=hscr)

        # ---------- out projection ----------
        for nt in range(NT):
            for doh in range(2):
                acc = ps_mm.tile([P, 512], f32, tag="mm")
                for kc in range(KT):
                    nc.tensor.matmul(acc, otfull[kc][:, ts(nt, P)],
                                     woutT[kc][:, ts(doh, 512)],
                                     start=(kc == 0), stop=(kc == KT - 1))
                ostage = work2.tile([P, 512], f32, tag="ostage")
                nc.vector.tensor_add(ostage, acc, b_bcast[:, ts(doh, 512)])
                nc.sync.dma_start(out=out_d[ts(nt, P), ts(doh, 512)], in_=ostage)

    nc.finalize()
    return nc


def kernel(x, mask, pos, W_qk, W_v, W_out, b_out):
    global _NC
    from concourse.bass_utils import run_bass_kernel_spmd

    if _NC is None:
        _NC = _build()

    x = np.ascontiguousarray(x, dtype=np.float32)
    pos = np.ascontiguousarray(pos, dtype=np.float32)
    maskf = np.concatenate(
        [np.ones((B, 1), np.float32), np.asarray(mask).astype(np.float32)], axis=1)
    W_qk = np.ascontiguousarray(W_qk, dtype=np.float32)
    W_v = np.ascontiguousarray(W_v, dtype=np.float32)
    W_out = np.ascontiguousarray(W_out, dtype=np.float32)
    b_out = np.ascontiguousarray(b_out, dtype=np.float32)

    in_maps = [
        {"x": x[b], "pos": pos[b], "maskf": maskf[b], "W_qk": W_qk,
         "W_v": W_v, "W_out": W_out, "b_out": b_out}
        for b in range(B)
    ]
    res = run_bass_kernel_spmd(_NC, in_maps, core_ids=list(range(B)))
    return np.stack([res.results[b]["out"] for b in range(B)]).astype(np.float32)



# revision 9
# speedup vs baseline: 1.4122x; 1.4122x over previous
"""Multi-head attention kernel for Trainium2, batch-parallel across 8 NeuronCores.

Reference (per batch element b, one core each):
  qk = x @ W_qk.T ; q,k = split(qk) ; v = x @ W_v.T
  q,k,v -> [h, n, d] ; q += pos_h ; k += pos_h
  S = q @ k.T * DIM**-0.5 ; mask = outer(m, m) ; masked -> -inf
  P = softmax(S) ; O = P @ v ; out = merge_heads(O) @ W_out.T + b_out

Device strategy (per core):
  - all layout work (transposes, bf16 casts, mask-derived tensors) done on
    HOST in numpy; the device receives x^T, pos^T, W_q^T, W_k^T, W_v^T,
    W_out^T in bf16 and streams pure matmuls.
  - scores computed TRANSPOSED per head pair: the two heads' K=64 score
    matmuls go to row groups 0-1 / 2-3 of the PE array (base partitions 0
    and 64) and run CONCURRENTLY (row tiling).
  - exp via one 1024-wide ACT per (pair, jt, ih) over both heads' scores in
    a [128, 2, 512] PSUM tile; the column mask folds into the per-partition
    exp bias, softmax row sums come from an appended ones-column in the PV
    matmul (V_aug = [V_h | 1], M=65).
  - per-pair attention is two i-half sweeps (ih=0,1) so both heads' PV
    accumulators fit one 2-bank PSUM tile; the softmax tail (1/s, row mask,
    masked-row blend with mean(V)) runs per (ih, head) off the PE path.
  - the next pair's q/k projections are interleaved into the attention
    units so the PE never waits on the ACT engine.
  - PSUM budget exactly 8 banks: score ring 2x2 + proj 2 + PV acc 2.
"""
import sys

sys.path.insert(0, "/opt/trn_rl_repo")

import numpy as np
import ml_dtypes
from contextlib import ExitStack

B, N, DIM, H = 8, 1024, 1024, 16
D = DIM // H          # 64
E = D + 1             # V_aug block (64 cols of V + ones column)
P = 128
NT = N // P           # 8 n-tiles
KT = DIM // P         # 8 k-tiles
NPAIR = H // 2        # 8 head pairs
SCALE = DIM ** (-0.5)
MB = 30.0             # mask bias magnitude: bias_j = 30*m - 30 in {0, -30}

_NC = None


def _build():
    import concourse.bacc as bacc
    import concourse.bass as bass
    import concourse.mybir as mybir
    import concourse.tile as tile

    f32 = mybir.dt.float32
    bf16 = mybir.dt.bfloat16
    AF = mybir.ActivationFunctionType
    OP = mybir.AluOpType
    ts = bass.ts

    nc = bacc.Bacc()
    xT_d = nc.declare_dram_parameter("xT", [DIM, N], bf16, isOutput=False)
    posT_d = nc.declare_dram_parameter("posT", [DIM, N], bf16, isOutput=False)
    wqT_d = nc.declare_dram_parameter("wqT", [DIM, DIM], bf16, isOutput=False)
    wkT_d = nc.declare_dram_parameter("wkT", [DIM, DIM], bf16, isOutput=False)
    wvT_d = nc.declare_dram_parameter("wvT", [DIM, DIM], bf16, isOutput=False)
    woT_d = nc.declare_dram_parameter("woT", [DIM, DIM], bf16, isOutput=False)
    biasj_d = nc.declare_dram_parameter("biasj", [P, NT], f32, isOutput=False)
    mcoll_d = nc.declare_dram_parameter("mcoll", [P, 16], f32, isOutput=False)
    omm_d = nc.declare_dram_parameter("omm", [N], f32, isOutput=False)
    b_d = nc.declare_dram_parameter("b_out", [DIM], f32, isOutput=False)
    out_d = nc.declare_dram_parameter("out", [N, DIM], f32, isOutput=True)

    with ExitStack() as ctx:
        tc = ctx.enter_context(tile.TileContext(nc))
        sing = ctx.enter_context(tc.tile_pool(name="sing", bufs=1))
        qk_pool = ctx.enter_context(tc.tile_pool(name="qk", bufs=2))
        expool = ctx.enter_context(tc.tile_pool(name="expool", bufs=3))
        tailp = ctx.enter_context(tc.tile_pool(name="tailp", bufs=2))
        ps_st = ctx.enter_context(tc.tile_pool(name="ps_st", bufs=2, space="PSUM"))
        ps_pj = ctx.enter_context(tc.tile_pool(name="ps_pj", bufs=1, space="PSUM"))
        ps_oa = ctx.enter_context(tc.tile_pool(name="ps_oa", bufs=1, space="PSUM"))

        # ---------- persistent SBUF ----------
        xT = sing.tile([P, KT, N], bf16, tag="xT")
        posT = sing.tile([P, KT, N], bf16, tag="posT")
        wqT = sing.tile([P, KT, DIM], bf16, tag="wqT")
        wkT = sing.tile([P, KT, DIM], bf16, tag="wkT")
        wvT = sing.tile([P, KT, DIM], bf16, tag="wvT")
        woT = sing.tile([P, KT, DIM], bf16, tag="woT")
        V_sb = [sing.tile([P, H * E], bf16, tag=f"V{nt}", name=f"V{nt}")
                for nt in range(NT)]
        otfull = [sing.tile([P, N], bf16, tag=f"otf{kc}", name=f"otf{kc}")
                  for kc in range(KT)]
        biasj = sing.tile([P, NT], f32, tag="biasj")
        mcoll = sing.tile([P, 16], f32, tag="mcoll")
        omm_row = sing.tile([1, N], f32, tag="omm_row")
        ommb = sing.tile([D, N], f32, tag="ommb")
        b_row = sing.tile([1, DIM], f32, tag="b_row")
        b_bcast = sing.tile([P, DIM], f32, tag="b_bcast")
        mean_sb = sing.tile([1, D * H], f32, tag="mean_sb")
        mean_cols = sing.tile([D, H], f32, tag="mean_cols")

        # ---------- input DMAs ----------
        for kc in range(KT):
            nc.sync.dma_start(out=xT[:, kc, :], in_=xT_d[ts(kc, P), :])
        for w_sb, w_d in ((wqT, wqT_d), (wkT, wkT_d)):
            for kc in range(KT):
                nc.sync.dma_start(out=w_sb[:, kc, :], in_=w_d[ts(kc, P), :])
        for kc in range(KT):
            nc.sync.dma_start(out=wvT[:, kc, :], in_=wvT_d[ts(kc, P), :])
        # scalar engine also hosts a DMA queue and is idle until attention:
        # route the later-needed loads there so issue time halves.
        for kc in range(KT):
            nc.scalar.dma_start(out=posT[:, kc, :], in_=posT_d[ts(kc, P), :])
        for kc in range(KT):
            nc.scalar.dma_start(out=woT[:, kc, :], in_=woT_d[ts(kc, P), :])
        nc.scalar.dma_start(out=biasj, in_=biasj_d[:, :])
        nc.scalar.dma_start(out=mcoll, in_=mcoll_d[:, :])
        nc.scalar.dma_start(out=omm_row, in_=omm_d[:])
        nc.scalar.dma_start(out=b_row, in_=b_d[:])

        # ---------- small prep (off-PE) ----------
        nc.gpsimd.partition_broadcast(ommb, omm_row)
        nc.gpsimd.partition_broadcast(b_bcast, b_row)
        const1 = sing.tile([P, H], f32, tag="const1")
        nc.vector.memset(const1, 1.0)
        for nt in range(NT):
            ones_ap = V_sb[nt].rearrange("p (h e) -> p h e", e=E)[:, :, D:E]
            nc.vector.tensor_copy(ones_ap.squeeze(), const1)
        constN = sing.tile([P, 1], f32, tag="constN")
        nc.vector.memset(constN, 1.0 / N)
        ones_col = sing.tile([P, 1], bf16, tag="ones_col")
        nc.vector.tensor_copy(ones_col, constN)

        # ---------- V = x @ W_v.T  (stored as [V_h | 1] x 16 heads) ----------
        for nt in range(NT):
            pool, tg = (ps_st, "st") if nt % 2 else (ps_pj, "pj")
            pj = pool.tile([P, 2, 512], f32, tag=tg)
            for dvh in range(2):
                for kc in range(KT):
                    nc.tensor.matmul(pj[:, dvh, :], xT[:, kc, ts(nt, P)],
                                     wvT[:, kc, ts(dvh, 512)],
                                     start=(kc == 0), stop=(kc == KT - 1))
            for dvh in range(2):
                dst = V_sb[nt][:, dvh * 8 * E: dvh * 8 * E + 8 * E].rearrange(
                    "p (h e) -> p h e", e=E)[:, :, 0:D]
                nc.vector.tensor_copy(
                    dst, pj[:, dvh, :].rearrange("p (h e) -> p h e", e=D))

        # ---------- mean over sequence of V_aug ----------
        # head-aligned chunks (7h, 7h, 2h); psum->sbuf copies write mean_sb
        # in (e, h) order so one flat DMA yields mean_cols[e, h].
        mt = ps_pj.tile([P, 2, 512], f32, tag="pj")
        mt2 = ps_st.tile([P, 2, 512], f32, tag="st")
        chunks = ((0, 7, mt[0:1, 0, :]), (7, 7, mt[0:1, 1, :]),
                  (14, 2, mt2[0:1, 0, :]))
        for h0, hn, dstp in chunks:
            for nt in range(NT):
                nc.tensor.matmul(dstp[:, 0:hn * E], ones_col,
                                 V_sb[nt][:, h0 * E:(h0 + hn) * E],
                                 start=(nt == 0), stop=(nt == NT - 1))
        mean_eh = mean_sb.rearrange("o (e h) -> o e h", h=H)  # [1, 64, 16]
        for h0, hn, dstp in chunks:
            nc.vector.tensor_copy(
                mean_eh[:, :, h0:h0 + hn].rearrange("o e h -> o h e"),
                dstp[:, 0:hn * E].rearrange("o (h e) -> o h e", e=E)[:, :, 0:D])
        nc.sync.dma_start(out=mean_cols, in_=mean_sb[0:1, 0:D * H])

        # ---------- projection groups (q/k for one pair) ----------
        def proj_ops(t):
            """Returns (ops, results): ops is a list of closures, each emits
            one instruction for the q/k projections of pair t."""
            qT_t = qk_pool.tile([P, N], bf16, tag="qT", name=f"qT{t}")
            kT_t = qk_pool.tile([P, N], bf16, tag="kT", name=f"kT{t}")
            ops = []
            state = {}

            def mk_alloc(which):
                def _op():
                    state[which] = ps_pj.tile([P, 2, 512], f32, tag="pj",
                                              name=f"pj{which}")
                return _op

            def mk_mm(which, w_sb, half, kc):
                def _op():
                    nc.tensor.matmul(state[which][:, half, :],
                                     w_sb[:, kc, ts(t, P)],
                                     xT[:, kc, ts(half, 512)],
                                     start=(kc == 0), stop=(kc == KT - 1))
                return _op

            def mk_tt(which, dstT, half):
                def _op():
                    nc.vector.tensor_add(dstT[:, ts(half, 512)],
                                         state[which][:, half, :],
                                         posT[:, t, ts(half, 512)])
                return _op

            for which, w_sb, dstT in (("q", wqT, qT_t), ("k", wkT, kT_t)):
                ops.append(mk_alloc(which))
                for half in range(2):
                    for kc in range(KT):
                        ops.append(mk_mm(which, w_sb, half, kc))
                for half in range(2):
                    ops.append(mk_tt(which, dstT, half))
            return ops, (qT_t, kT_t)

        # pair 0 projections up front
        ops0, qk0 = proj_ops(0)
        for op in ops0:
            op()

        # ---------- per-pair attention ----------
        cur_qk = qk0
        for t in range(NPAIR):
            qT_t, kT_t = cur_qk
            if t + 1 < NPAIR:
                pend, cur_qk = proj_ops(t + 1)
            else:
                pend, cur_qk = [], None
            pend = list(pend)

            for ih in range(2):
                oa = ps_oa.tile([E, 2, 512], f32, tag="oa", name=f"oa{t}_{ih}")
                for jt in range(NT):
                    st = ps_st.tile([P, 2, 512], f32, tag="st")
                    for hs in range(2):
                        nc.tensor.matmul(st[:, hs, :],
                                         kT_t[ts(hs, D), ts(jt, P)],
                                         qT_t[ts(hs, D), ts(ih, 512)],
                                         start=True, stop=True)
                    ex = expool.tile([P, 2, 512], bf16, tag="ex")
                    nc.scalar.activation(ex, st, AF.Exp,
                                         bias=biasj[:, jt:jt + 1], scale=SCALE)
                    for hs in range(2):
                        h = 2 * t + hs
                        nc.tensor.matmul(oa[:, hs, :],
                                         V_sb[jt][:, h * E:(h + 1) * E],
                                         ex[:, hs, :],
                                         start=(jt == 0), stop=(jt == NT - 1))
                    # interleave projection work for pair t+1
                    budget = 3 if jt >= 6 else 2
                    for _ in range(budget):
                        if pend:
                            pend.pop(0)()

                # ---- softmax tail for (t, ih), both heads ----
                oaS = tailp.tile([E, 2, 512], f32, tag="oaS")
                nc.vector.tensor_copy(oaS, oa)     # frees the PSUM acc
                # collect s rows into [p, c] layout: s_coll[p, hs, c] =
                # s_hs[p*4 + c] (both APs flatten row-major -> streams match)
                s_coll = tailp.tile([P, 2, 4], f32, tag="s_coll")
                for hs in range(2):
                    nc.sync.dma_start(out=s_coll[:, hs, :],
                                      in_=oaS[D:D + 1, hs, :])
                r_coll = tailp.tile([P, 2, 4], f32, tag="r_coll")
                nc.vector.reciprocal(r_coll, s_coll)
                nc.vector.tensor_mul(
                    r_coll, r_coll,
                    mcoll[:, ih * 8:(ih + 1) * 8].rearrange(
                        "p (h c) -> p h c", c=4))
                for hs in range(2):
                    h = 2 * t + hs
                    rm_row = tailp.tile([1, 512], f32, tag=f"rm{hs}")
                    nc.sync.dma_start(
                        out=rm_row.rearrange("o (p c) -> o p c", c=4),
                        in_=r_coll[:, hs, :],
                    )
                    rmb = tailp.tile([D, 512], f32, tag=f"rmb{hs}")
                    nc.gpsimd.partition_broadcast(rmb, rm_row)
                    t1 = tailp.tile([D, 512], f32, tag=f"t1{hs}")
                    nc.vector.tensor_mul(t1, oaS[0:D, hs, :], rmb)
                    if hs == 0:
                        nc.vector.scalar_tensor_tensor(
                            otfull[t][0:D, ts(ih, 512)],
                            ommb[:, ts(ih, 512)], mean_cols[:, h:h + 1], t1,
                            OP.mult, OP.add)
                    else:
                        hscr = tailp.tile([D, 512], bf16, tag="hscr")
                        nc.vector.scalar_tensor_tensor(
                            hscr, ommb[:, ts(ih, 512)],
                            mean_cols[:, h:h + 1], t1, OP.mult, OP.add)
                        nc.sync.dma_start(
                            out=otfull[t][D:P, ts(ih, 512)], in_=hscr)
            # any leftover projection ops
            for op in pend:
                op()

        # ---------- out projection ----------
        for nt in range(NT):
            pool, tg = (ps_st, "st") if nt % 2 else (ps_pj, "pj")
            pj = pool.tile([P, 2, 512], f32, tag=tg)
            for doh in range(2):
                for kc in range(KT):
                    nc.tensor.matmul(pj[:, doh, :], otfull[kc][:, ts(nt, P)],
                                     woT[:, kc, ts(doh, 512)],
                                     start=(kc == 0), stop=(kc == KT - 1))
            for doh in range(2):
                ostage = tailp.tile([P, 512], f32, tag="ostage")
                nc.vector.tensor_add(ostage, pj[:, doh, :],
                                     b_bcast[:, ts(doh, 512)])
                nc.sync.dma_start(out=out_d[ts(nt, P), ts(doh, 512)],
                                  in_=ostage)

    nc.finalize()
    return nc


def _host_prep(x, mask, pos, W_qk, W_v, W_out, b_out):
    bf = ml_dtypes.bfloat16
    x = np.ascontiguousarray(x, dtype=np.float32)
    pos = np.ascontiguousarray(pos, dtype=np.float32)
    W_qk = np.asarray(W_qk, dtype=np.float32)
    maskf = np.concatenate(
        [np.ones((B, 1), np.float32), np.asarray(mask).astype(np.float32)],
        axis=1)                                        # [B, N]
    wqT = np.ascontiguousarray(W_qk[:DIM].T.astype(bf))
    wkT = np.ascontiguousarray(W_qk[DIM:].T.astype(bf))
    wvT = np.ascontiguousarray(np.asarray(W_v, np.float32).T.astype(bf))
    woT = np.ascontiguousarray(np.asarray(W_out, np.float32).T.astype(bf))
    b_out = np.ascontiguousarray(b_out, dtype=np.float32)

    in_maps = []
    for b in range(B):
        m = maskf[b]
        biasj = np.ascontiguousarray(
            (MB * m - MB).reshape(NT, P).T)            # [p, jt]
        # mcoll[p, ih*8 + hs*4 + c] = m[ih*512 + p*4 + c] (dup for both heads)
        mc = m.reshape(2, P, 4)                        # [ih, p, c]
        mcoll = np.ascontiguousarray(
            np.stack([mc[0], mc[0], mc[1], mc[1]],
                     axis=1).reshape(P, 16))
        in_maps.append({
            "xT": np.ascontiguousarray(x[b].T.astype(bf)),
            "posT": np.ascontiguousarray(pos[b].T.astype(bf)),
            "wqT": wqT, "wkT": wkT, "wvT": wvT, "woT": woT,
            "biasj": biasj.astype(np.float32),
            "mcoll": mcoll.astype(np.float32),
            "omm": np.ascontiguousarray(1.0 - m),
            "b_out": b_out,
        })
    return in_maps


def kernel(x, mask, pos, W_qk, W_v, W_out, b_out):
    global _NC
    from concourse.bass_utils import run_bass_kernel_spmd

    if _NC is None:
        _NC = _build()

    in_maps = _host_prep(x, mask, pos, W_qk, W_v, W_out, b_out)
    res = run_bass_kernel_spmd(_NC, in_maps, core_ids=list(range(B)))
    return np.stack([res.results[b]["out"] for b in range(B)]).astype(np.float32)


# revision 14
# speedup vs baseline: 1.5654x; 1.1085x over previous
"""Multi-head attention kernel for Trainium2, batch-parallel across 8 NeuronCores.

Reference (per batch element b, one core each):
  qk = x @ W_qk.T ; q,k = split(qk) ; v = x @ W_v.T
  q,k,v -> [h, n, d] ; q += pos_h ; k += pos_h
  S = q @ k.T * DIM**-0.5 ; mask = outer(m, m) ; masked -> -inf
  P = softmax(S) ; O = P @ v ; out = merge_heads(O) @ W_out.T + b_out

Device strategy (per core):
  - all layout work (transposes, bf16 casts, mask-derived tensors) done on
    HOST in numpy; the device receives x^T, pos^T, W_q^T, W_k^T, W_v^T,
    W_out^T in bf16 and streams pure matmuls.
  - scores computed TRANSPOSED per head pair: the two heads' K=64 score
    matmuls go to row groups 0-1 / 2-3 of the PE array (base partitions 0
    and 64) and run CONCURRENTLY (row tiling).
  - exp via one 1024-wide ACT per (pair, jt, ih) over both heads' scores in
    a [128, 2, 512] PSUM tile; the column mask folds into the per-partition
    exp bias, softmax row sums come from an appended ones-column in the PV
    matmul (V_aug = [V_h | 1], M=65).
  - per-pair attention is two i-half sweeps (ih=0,1) so both heads' PV
    accumulators fit one 2-bank PSUM tile; the softmax tail (1/s, row mask,
    masked-row blend with mean(V)) runs per (ih, head) off the PE path.
  - the next pair's q/k projections are interleaved into the attention
    units so the PE never waits on the ACT engine.
  - PSUM budget exactly 8 banks: score ring 2x2 + proj 2 + PV acc 2.
"""
import sys

sys.path.insert(0, "/opt/trn_rl_repo")

import numpy as np
import ml_dtypes
from contextlib import ExitStack

B, N, DIM, H = 8, 1024, 1024, 16
D = DIM // H          # 64
E = D + 1             # V_aug block (64 cols of V + ones column)
P = 128
NT = N // P           # 8 n-tiles
KT = DIM // P         # 8 k-tiles
NPAIR = H // 2        # 8 head pairs
SCALE = DIM ** (-0.5)
MB = 30.0             # mask bias magnitude: bias_j = 30*m - 30 in {0, -30}

_NC = None


def _build():
    import concourse.bacc as bacc
    import concourse.bass as bass
    import concourse.mybir as mybir
    import concourse.tile as tile

    f32 = mybir.dt.float32
    bf16 = mybir.dt.bfloat16
    AF = mybir.ActivationFunctionType
    OP = mybir.AluOpType
    ts = bass.ts

    nc = bacc.Bacc()
    xT_d = nc.declare_dram_parameter("xT", [DIM, N], bf16, isOutput=False)
    posT_d = nc.declare_dram_parameter("posT", [DIM, N], bf16, isOutput=False)
    wqT_d = nc.declare_dram_parameter("wqT", [DIM, DIM], bf16, isOutput=False)
    wkT_d = nc.declare_dram_parameter("wkT", [DIM, DIM], bf16, isOutput=False)
    wvT_d = nc.declare_dram_parameter("wvT", [DIM, DIM], bf16, isOutput=False)
    woT_d = nc.declare_dram_parameter("woT", [DIM, DIM], bf16, isOutput=False)
    biasj_d = nc.declare_dram_parameter("biasj", [P, NT], f32, isOutput=False)
    mcoll_d = nc.declare_dram_parameter("mcoll", [P, 16], f32, isOutput=False)
    omm_d = nc.declare_dram_parameter("omm", [N], f32, isOutput=False)
    b_d = nc.declare_dram_parameter("b_out", [DIM], f32, isOutput=False)
    out_d = nc.declare_dram_parameter("out", [N, DIM], f32, isOutput=True)

    with ExitStack() as ctx:
        tc = ctx.enter_context(tile.TileContext(nc))
        sing = ctx.enter_context(tc.tile_pool(name="sing", bufs=1))
        qk_pool = ctx.enter_context(tc.tile_pool(name="qk", bufs=2))
        expool = ctx.enter_context(tc.tile_pool(name="expool", bufs=3))
        tailp = ctx.enter_context(tc.tile_pool(name="tailp", bufs=2))
        ps_st = ctx.enter_context(tc.tile_pool(name="ps_st", bufs=2, space="PSUM"))
        ps_pj = ctx.enter_context(tc.tile_pool(name="ps_pj", bufs=1, space="PSUM"))
        ps_oa = ctx.enter_context(tc.tile_pool(name="ps_oa", bufs=1, space="PSUM"))

        # ---------- persistent SBUF ----------
        xT = sing.tile([P, KT, N], bf16, tag="xT")
        posT = sing.tile([P, KT, N], bf16, tag="posT")
        wqT = sing.tile([P, KT, DIM], bf16, tag="wqT")
        wkT = sing.tile([P, KT, DIM], bf16, tag="wkT")
        wvT = sing.tile([P, KT, DIM], bf16, tag="wvT")
        woT = sing.tile([P, KT, DIM], bf16, tag="woT")
        V_sb = [sing.tile([P, H * E], bf16, tag=f"V{nt}", name=f"V{nt}")
                for nt in range(NT)]
        otfull = [sing.tile([P, N], bf16, tag=f"otf{kc}", name=f"otf{kc}")
                  for kc in range(KT)]
        biasj = sing.tile([P, NT], f32, tag="biasj")
        mcoll = sing.tile([P, 16], f32, tag="mcoll")
        omm_row = sing.tile([1, N], f32, tag="omm_row")
        ommb = sing.tile([D, N], f32, tag="ommb")
        b_row = sing.tile([1, DIM], f32, tag="b_row")
        b_bcast = sing.tile([P, DIM], f32, tag="b_bcast")
        mean_sb = sing.tile([1, D * H], f32, tag="mean_sb")
        mean_cols = sing.tile([D, H], f32, tag="mean_cols")

        # ---------- input DMAs ----------
        # Two parallel hwdge queues (sync + scalar), each ~190 GB/s. Order
        # so V-proj deps (wvT + xT) land first, then pair-0's proj deps.
        # xT loads are per-token-block so V-proj group nt waits only on
        # block nt; wq/wk loads are per-pair slices.
        xTd_v = xT_d.rearrange("(kc p) t -> p kc t", p=P)
        wq_v = wqT_d.rearrange("(kc p) d -> p kc d", p=P)
        wk_v = wkT_d.rearrange("(kc p) d -> p kc d", p=P)
        for kc in range(0, KT, 2):
            nc.sync.dma_start(out=wvT[:, kc, :], in_=wvT_d[ts(kc, P), :])
        for kc in range(1, KT, 2):
            nc.scalar.dma_start(out=wvT[:, kc, :], in_=wvT_d[ts(kc, P), :])
        for nt in range(0, NT, 2):
            nc.sync.dma_start(out=xT[:, :, ts(nt, P)], in_=xTd_v[:, :, ts(nt, P)])
        for nt in range(1, NT, 2):
            nc.scalar.dma_start(out=xT[:, :, ts(nt, P)], in_=xTd_v[:, :, ts(nt, P)])
        nc.scalar.dma_start(out=biasj, in_=biasj_d[:, :])
        nc.scalar.dma_start(out=mcoll, in_=mcoll_d[:, :])
        nc.scalar.dma_start(out=omm_row, in_=omm_d[:])
        nc.scalar.dma_start(out=b_row, in_=b_d[:])
        for t in range(NPAIR):
            nc.sync.dma_start(out=wqT[:, :, ts(t, P)], in_=wq_v[:, :, ts(t, P)])
            nc.scalar.dma_start(out=wkT[:, :, ts(t, P)], in_=wk_v[:, :, ts(t, P)])
            nc.scalar.dma_start(out=posT[:, t, :], in_=posT_d[ts(t, P), :])
        for kc in range(0, KT, 2):
            nc.sync.dma_start(out=woT[:, kc, :], in_=woT_d[ts(kc, P), :])
        for kc in range(1, KT, 2):
            nc.scalar.dma_start(out=woT[:, kc, :], in_=woT_d[ts(kc, P), :])

        # ---------- small prep (off-PE) ----------
        nc.gpsimd.partition_broadcast(ommb, omm_row)
        nc.gpsimd.partition_broadcast(b_bcast, b_row)
        const1 = sing.tile([P, H], f32, tag="const1")
        nc.vector.memset(const1, 1.0)
        for nt in range(NT):
            ones_ap = V_sb[nt].rearrange("p (h e) -> p h e", e=E)[:, :, D:E]
            nc.vector.tensor_copy(ones_ap.squeeze(), const1)
        constN = sing.tile([P, 1], f32, tag="constN")
        nc.vector.memset(constN, 1.0 / N)
        ones_col = sing.tile([P, 1], bf16, tag="ones_col")
        nc.vector.tensor_copy(ones_col, constN)

        # ---------- V = x @ W_v.T  (stored as [V_h | 1] x 16 heads) ----------
        for nt in range(NT):
            pool, tg = (ps_st, "st") if nt % 2 else (ps_pj, "pj")
            pj = pool.tile([P, 2, 512], f32, tag=tg)
            # kc-major so consecutive matmuls share the stationary operand
            for kc in range(KT):
                for dvh in range(2):
                    nc.tensor.matmul(pj[:, dvh, :], xT[:, kc, ts(nt, P)],
                                     wvT[:, kc, ts(dvh, 512)],
                                     start=(kc == 0), stop=(kc == KT - 1))
            for dvh in range(2):
                dst = V_sb[nt][:, dvh * 8 * E: dvh * 8 * E + 8 * E].rearrange(
                    "p (h e) -> p h e", e=E)[:, :, 0:D]
                nc.vector.tensor_copy(
                    dst, pj[:, dvh, :].rearrange("p (h e) -> p h e", e=D))

        # ---------- mean over sequence of V_aug ----------
        # head-aligned chunks (7h, 7h, 2h); psum->sbuf copies write mean_sb
        # in (e, h) order so one flat DMA yields mean_cols[e, h].
        mt = ps_pj.tile([P, 2, 512], f32, tag="pj")
        mt2 = ps_st.tile([P, 2, 512], f32, tag="st")
        chunks = ((0, 7, mt[0:1, 0, :]), (7, 7, mt[0:1, 1, :]),
                  (14, 2, mt2[0:1, 0, :]))
        for h0, hn, dstp in chunks:
            for nt in range(NT):
                nc.tensor.matmul(dstp[:, 0:hn * E], ones_col,
                                 V_sb[nt][:, h0 * E:(h0 + hn) * E],
                                 start=(nt == 0), stop=(nt == NT - 1))
        mean_eh = mean_sb.rearrange("o (e h) -> o e h", h=H)  # [1, 64, 16]
        for h0, hn, dstp in chunks:
            nc.vector.tensor_copy(
                mean_eh[:, :, h0:h0 + hn].rearrange("o e h -> o h e"),
                dstp[:, 0:hn * E].rearrange("o (h e) -> o h e", e=E)[:, :, 0:D])
        nc.sync.dma_start(out=mean_cols, in_=mean_sb[0:1, 0:D * H])

        # ---------- projection groups (q/k for one pair) ----------
        def proj_ops(t):
            """Returns (ops, results): ops is a list of closures, each emits
            one instruction for the q/k projections of pair t."""
            qT_t = qk_pool.tile([P, N], bf16, tag="qT", name=f"qT{t}")
            kT_t = qk_pool.tile([P, N], bf16, tag="kT", name=f"kT{t}")
            ops = []
            state = {}

            def mk_alloc(which):
                def _op():
                    state[which] = ps_pj.tile([P, 2, 512], f32, tag="pj",
                                              name=f"pj{which}")
                return _op

            def mk_mm(which, w_sb, half, kc):
                def _op():
                    nc.tensor.matmul(state[which][:, half, :],
                                     w_sb[:, kc, ts(t, P)],
                                     xT[:, kc, ts(half, 512)],
                                     start=(kc == 0), stop=(kc == KT - 1))
                return _op

            def mk_tt(which, dstT, half):
                def _op():
                    nc.vector.tensor_add(dstT[:, ts(half, 512)],
                                         state[which][:, half, :],
                                         posT[:, t, ts(half, 512)])
                return _op

            for which, w_sb, dstT in (("q", wqT, qT_t), ("k", wkT, kT_t)):
                ops.append(mk_alloc(which))
                # kc-major: both halves reuse the same stationary weights
                for kc in range(KT):
                    for half in range(2):
                        ops.append(mk_mm(which, w_sb, half, kc))
                for half in range(2):
                    ops.append(mk_tt(which, dstT, half))
            return ops, (qT_t, kT_t)

        # pair 0 projections up front
        ops0, qk0 = proj_ops(0)
        for op in ops0:
            op()

        # ---------- per-pair attention ----------
        cur_qk = qk0
        for t in range(NPAIR):
            qT_t, kT_t = cur_qk
            if t + 1 < NPAIR:
                pend, cur_qk = proj_ops(t + 1)
            else:
                pend, cur_qk = [], None
            pend = list(pend)

            for ih in range(2):
                oa = ps_oa.tile([E, 2, 512], f32, tag="oa", name=f"oa{t}_{ih}")
                exs = [None] * NT

                def emit_pv(jt):
                    for hs in range(2):
                        h = 2 * t + hs
                        nc.tensor.matmul(oa[:, hs, :],
                                         V_sb[jt][:, h * E:(h + 1) * E],
                                         exs[jt][:, hs, :],
                                         start=(jt == 0), stop=(jt == NT - 1))

                for jt in range(NT):
                    st = ps_st.tile([P, 2, 512], f32, tag="st")
                    for hs in range(2):
                        nc.tensor.matmul(st[:, hs, :],
                                         kT_t[ts(hs, D), ts(jt, P)],
                                         qT_t[ts(hs, D), ts(ih, 512)],
                                         start=True, stop=True)
                    ex = expool.tile([P, 2, 512], bf16, tag="ex")
                    nc.scalar.activation(ex, st, AF.Exp,
                                         bias=biasj[:, jt:jt + 1], scale=SCALE)
                    exs[jt] = ex
                    # PV lags one jt behind so the exp has a full unit of
                    # slack before the PE needs its output.
                    if jt > 0:
                        emit_pv(jt - 1)
                    budget = 3 if jt >= 6 else 2
                    for _ in range(budget):
                        if pend:
                            pend.pop(0)()
                emit_pv(NT - 1)

                # ---- softmax tail for (t, ih), both heads ----
                oaS = tailp.tile([E, 2, 512], f32, tag="oaS")
                nc.vector.tensor_copy(oaS, oa)     # frees the PSUM acc
                # collect s rows into [p, c] layout: s_coll[p, hs, c] =
                # s_hs[p*4 + c] (both APs flatten row-major -> streams match)
                s_coll = tailp.tile([P, 2, 4], f32, tag="s_coll")
                for hs in range(2):
                    nc.sync.dma_start(out=s_coll[:, hs, :],
                                      in_=oaS[D:D + 1, hs, :])
                r_coll = tailp.tile([P, 2, 4], f32, tag="r_coll")
                nc.vector.reciprocal(r_coll, s_coll)
                nc.vector.tensor_mul(
                    r_coll, r_coll,
                    mcoll[:, ih * 8:(ih + 1) * 8].rearrange(
                        "p (h c) -> p h c", c=4))
                for hs in range(2):
                    h = 2 * t + hs
                    rm_row = tailp.tile([1, 512], f32, tag=f"rm{hs}")
                    nc.sync.dma_start(
                        out=rm_row.rearrange("o (p c) -> o p c", c=4),
                        in_=r_coll[:, hs, :],
                    )
                    rmb = tailp.tile([D, 512], f32, tag=f"rmb{hs}")
                    nc.gpsimd.partition_broadcast(rmb, rm_row)
                    t1 = tailp.tile([D, 512], f32, tag=f"t1{hs}")
                    nc.vector.tensor_mul(t1, oaS[0:D, hs, :], rmb)
                    if hs == 0:
                        nc.vector.scalar_tensor_tensor(
                            otfull[t][0:D, ts(ih, 512)],
                            ommb[:, ts(ih, 512)], mean_cols[:, h:h + 1], t1,
                            OP.mult, OP.add)
                    else:
                        hscr = tailp.tile([D, 512], bf16, tag="hscr")
                        nc.vector.scalar_tensor_tensor(
                            hscr, ommb[:, ts(ih, 512)],
                            mean_cols[:, h:h + 1], t1, OP.mult, OP.add)
                        nc.sync.dma_start(
                            out=otfull[t][D:P, ts(ih, 512)], in_=hscr)
            # any leftover projection ops
            for op in pend:
                op()

        # ---------- out projection ----------
        for nt in range(NT):
            pool, tg = (ps_st, "st") if nt % 2 else (ps_pj, "pj")
            pj = pool.tile([P, 2, 512], f32, tag=tg)
            for kc in range(KT):
                for doh in range(2):
                    nc.tensor.matmul(pj[:, doh, :], otfull[kc][:, ts(nt, P)],
                                     woT[:, kc, ts(doh, 512)],
                                     start=(kc == 0), stop=(kc == KT - 1))
            for doh in range(2):
                ostage = tailp.tile([P, 512], f32, tag="ostage")
                nc.vector.tensor_add(ostage, pj[:, doh, :],
                                     b_bcast[:, ts(doh, 512)])
                nc.sync.dma_start(out=out_d[ts(nt, P), ts(doh, 512)],
                                  in_=ostage)

    nc.finalize()
    return nc


def _host_prep(x, mask, pos, W_qk, W_v, W_out, b_out):
    bf = ml_dtypes.bfloat16
    x = np.ascontiguousarray(x, dtype=np.float32)
    pos = np.ascontiguousarray(pos, dtype=np.float32)
    W_qk = np.asarray(W_qk, dtype=np.float32)
    maskf = np.concatenate(
        [np.ones((B, 1), np.float32), np.asarray(mask).astype(np.float32)],
        axis=1)                                        # [B, N]
    wqT = np.ascontiguousarray(W_qk[:DIM].T.astype(bf))
    wkT = np.ascontiguousarray(W_qk[DIM:].T.astype(bf))
    wvT = np.ascontiguousarray(np.asarray(W_v, np.float32).T.astype(bf))
    woT = np.ascontiguousarray(np.asarray(W_out, np.float32).T.astype(bf))
    b_out = np.ascontiguousarray(b_out, dtype=np.float32)

    in_maps = []
    for b in range(B):
        m = maskf[b]
        biasj = np.ascontiguousarray(
            (MB * m - MB).reshape(NT, P).T)            # [p, jt]
        # mcoll[p, ih*8 + hs*4 + c] = m[ih*512 + p*4 + c] (dup for both heads)
        mc = m.reshape(2, P, 4)                        # [ih, p, c]
        mcoll = np.ascontiguousarray(
            np.stack([mc[0], mc[0], mc[1], mc[1]],
                     axis=1).reshape(P, 16))
        in_maps.append({
            "xT": np.ascontiguousarray(x[b].T.astype(bf)),
            "posT": np.ascontiguousarray(pos[b].T.astype(bf)),
            "wqT": wqT, "wkT": wkT, "wvT": wvT, "woT": woT,
            "biasj": biasj.astype(np.float32),
            "mcoll": mcoll.astype(np.float32),
            "omm": np.ascontiguousarray(1.0 - m),
            "b_out": b_out,
        })
    return in_maps


def kernel(x, mask, pos, W_qk, W_v, W_out, b_out):
    global _NC
    from concourse.bass_utils import run_bass_kernel_spmd

    if _NC is None:
        _NC = _build()

    in_maps = _host_prep(x, mask, pos, W_qk, W_v, W_out, b_out)
    res = run_bass_kernel_spmd(_NC, in_maps, core_ids=list(range(B)))
    return np.stack([res.results[b]["out"] for b in range(B)]).astype(np.float32)


# revision 20
# speedup vs baseline: 1.6670x; 1.0649x over previous
"""Multi-head attention kernel for Trainium2, batch-parallel across 8 NeuronCores.

Reference (per batch element b, one core each):
  qk = x @ W_qk.T ; q,k = split(qk) ; v = x @ W_v.T
  q,k,v -> [h, n, d] ; q += pos_h ; k += pos_h
  S = q @ k.T * DIM**-0.5 ; mask = outer(m, m) ; masked -> -inf
  P = softmax(S) ; O = P @ v ; out = merge_heads(O) @ W_out.T + b_out

Device strategy (per core):
  - all layout work (transposes, bf16 casts, mask-derived tensors) done on
    HOST in numpy; the device receives x^T, pos^T, W_q^T, W_k^T, W_v^T,
    W_out^T in bf16 and streams pure matmuls.
  - scores computed TRANSPOSED per head pair: the two heads' K=64 score
    matmuls go to row groups 0-1 / 2-3 of the PE array (base partitions 0
    and 64) and run CONCURRENTLY (row tiling).
  - exp via one 1024-wide ACT per (pair, jt, ih) over both heads' scores in
    a [128, 2, 512] PSUM tile; the column mask folds into the per-partition
    exp bias, softmax row sums come from an appended ones-column in the PV
    matmul (V_aug = [V_h | 1], M=65).
  - per-pair attention is two i-half sweeps (ih=0,1) so both heads' PV
    accumulators fit one 2-bank PSUM tile; the softmax tail (1/s, row mask,
    masked-row blend with mean(V)) runs per (ih, head) off the PE path.
  - the next pair's q/k projections are interleaved into the attention
    units so the PE never waits on the ACT engine.
  - PSUM budget exactly 8 banks: score ring 2x2 + proj 2 + PV acc 2.
"""
import sys

sys.path.insert(0, "/opt/trn_rl_repo")

import numpy as np
import ml_dtypes
from contextlib import ExitStack

B, N, DIM, H = 8, 1024, 1024, 16
D = DIM // H          # 64
E = D + 1             # V_aug block (64 cols of V + ones column)
P = 128
NT = N // P           # 8 n-tiles
KT = DIM // P         # 8 k-tiles
NPAIR = H // 2        # 8 head pairs
SCALE = DIM ** (-0.5)
MB = 30.0             # mask bias magnitude: bias_j = 30*m - 30 in {0, -30}

_NC = None


def _build():
    import concourse.bacc as bacc
    import concourse.bass as bass
    import concourse.mybir as mybir
    import concourse.tile as tile

    f32 = mybir.dt.float32
    bf16 = mybir.dt.bfloat16
    AF = mybir.ActivationFunctionType
    OP = mybir.AluOpType
    ts = bass.ts

    nc = bacc.Bacc()
    xT_d = nc.declare_dram_parameter("xT", [DIM, N], bf16, isOutput=False)
    posT_d = nc.declare_dram_parameter("posT", [DIM, N], bf16, isOutput=False)
    wqT_d = nc.declare_dram_parameter("wqT", [DIM, DIM], bf16, isOutput=False)
    wkT_d = nc.declare_dram_parameter("wkT", [DIM, DIM], bf16, isOutput=False)
    wvT_d = nc.declare_dram_parameter("wvT", [DIM, DIM], bf16, isOutput=False)
    woT_d = nc.declare_dram_parameter("woT", [DIM, DIM], bf16, isOutput=False)
    biasj_d = nc.declare_dram_parameter("biasj", [P, NT], f32, isOutput=False)
    mcoll_d = nc.declare_dram_parameter("mcoll", [P, 16], f32, isOutput=False)
    omm_d = nc.declare_dram_parameter("omm", [N], f32, isOutput=False)
    b_d = nc.declare_dram_parameter("b_out", [DIM], f32, isOutput=False)
    out_d = nc.declare_dram_parameter("out", [N, DIM], f32, isOutput=True)

    with ExitStack() as ctx:
        tc = ctx.enter_context(tile.TileContext(nc))
        sing = ctx.enter_context(tc.tile_pool(name="sing", bufs=1))
        qk_pool = ctx.enter_context(tc.tile_pool(name="qk", bufs=2))
        expool = ctx.enter_context(tc.tile_pool(name="expool", bufs=3))
        tailp = ctx.enter_context(tc.tile_pool(name="tailp", bufs=2))
        ps_st = ctx.enter_context(tc.tile_pool(name="ps_st", bufs=2, space="PSUM"))
        ps_pj = ctx.enter_context(tc.tile_pool(name="ps_pj", bufs=1, space="PSUM"))
        ps_oa = ctx.enter_context(tc.tile_pool(name="ps_oa", bufs=1, space="PSUM"))

        # ---------- persistent SBUF ----------
        xT = sing.tile([P, KT, N], bf16, tag="xT")
        posT = sing.tile([P, KT, N], bf16, tag="posT")
        wqT = sing.tile([P, KT, DIM], bf16, tag="wqT")
        wkT = sing.tile([P, KT, DIM], bf16, tag="wkT")
        wvT = sing.tile([P, KT, DIM], bf16, tag="wvT")
        woT = sing.tile([P, KT, DIM], bf16, tag="woT")
        V_sb = [sing.tile([P, H * E], bf16, tag=f"V{nt}", name=f"V{nt}")
                for nt in range(NT)]
        otfull = [sing.tile([P, N], bf16, tag=f"otf{kc}", name=f"otf{kc}")
                  for kc in range(KT)]
        biasj = sing.tile([P, NT], f32, tag="biasj")
        mcoll = sing.tile([P, 16], f32, tag="mcoll")
        omm_row = sing.tile([1, N], f32, tag="omm_row")
        ommb = sing.tile([D, N], f32, tag="ommb")
        b_row = sing.tile([1, DIM], f32, tag="b_row")
        b_bcast = sing.tile([P, DIM], f32, tag="b_bcast")
        mean_sb = sing.tile([1, D * H], f32, tag="mean_sb")
        mean_cols = sing.tile([D, H], f32, tag="mean_cols")

        # ---------- input DMAs ----------
        # Two parallel hwdge queues (sync + scalar), each ~190 GB/s. Order
        # so V-proj deps (wvT + xT) land first, then pair-0's proj deps.
        # xT loads are per-token-block so V-proj group nt waits only on
        # block nt; wq/wk loads are per-pair slices.
        xTd_v = xT_d.rearrange("(kc p) t -> p kc t", p=P)
        wq_v = wqT_d.rearrange("(kc p) d -> p kc d", p=P)
        wk_v = wkT_d.rearrange("(kc p) d -> p kc d", p=P)
        for kc in range(0, KT, 2):
            nc.sync.dma_start(out=wvT[:, kc, :], in_=wvT_d[ts(kc, P), :])
        for kc in range(1, KT, 2):
            nc.scalar.dma_start(out=wvT[:, kc, :], in_=wvT_d[ts(kc, P), :])
        for nt in range(0, NT, 2):
            nc.sync.dma_start(out=xT[:, :, ts(nt, P)], in_=xTd_v[:, :, ts(nt, P)])
        for nt in range(1, NT, 2):
            nc.scalar.dma_start(out=xT[:, :, ts(nt, P)], in_=xTd_v[:, :, ts(nt, P)])
        nc.scalar.dma_start(out=biasj, in_=biasj_d[:, :])
        nc.scalar.dma_start(out=mcoll, in_=mcoll_d[:, :])
        nc.scalar.dma_start(out=omm_row, in_=omm_d[:])
        nc.scalar.dma_start(out=b_row, in_=b_d[:])
        for t in range(NPAIR):
            nc.sync.dma_start(out=wqT[:, :, ts(t, P)], in_=wq_v[:, :, ts(t, P)])
            nc.scalar.dma_start(out=wkT[:, :, ts(t, P)], in_=wk_v[:, :, ts(t, P)])
            nc.scalar.dma_start(out=posT[:, t, :], in_=posT_d[ts(t, P), :])
        for kc in range(0, KT, 2):
            nc.sync.dma_start(out=woT[:, kc, :], in_=woT_d[ts(kc, P), :])
        for kc in range(1, KT, 2):
            nc.scalar.dma_start(out=woT[:, kc, :], in_=woT_d[ts(kc, P), :])

        # ---------- small prep (off-PE) ----------
        nc.gpsimd.partition_broadcast(ommb, omm_row)
        nc.gpsimd.partition_broadcast(b_bcast, b_row)
        const1 = sing.tile([P, H], f32, tag="const1")
        nc.vector.memset(const1, 1.0)
        for nt in range(NT):
            ones_ap = V_sb[nt].rearrange("p (h e) -> p h e", e=E)[:, :, D:E]
            nc.vector.tensor_copy(ones_ap.squeeze(), const1)
        constN = sing.tile([P, 1], f32, tag="constN")
        nc.vector.memset(constN, 1.0 / N)
        ones_col = sing.tile([P, 1], bf16, tag="ones_col")
        nc.vector.tensor_copy(ones_col, constN)

        # ---------- V = x @ W_v.T  (stored as [V_h | 1] x 16 heads) ----------
        for nt in range(NT):
            pool, tg = (ps_st, "st") if nt % 2 else (ps_pj, "pj")
            pj = pool.tile([P, 2, 512], f32, tag=tg)
            # kc-major so consecutive matmuls share the stationary operand
            for kc in range(KT):
                for dvh in range(2):
                    nc.tensor.matmul(pj[:, dvh, :], xT[:, kc, ts(nt, P)],
                                     wvT[:, kc, ts(dvh, 512)],
                                     start=(kc == 0), stop=(kc == KT - 1))
            for dvh in range(2):
                dst = V_sb[nt][:, dvh * 8 * E: dvh * 8 * E + 8 * E].rearrange(
                    "p (h e) -> p h e", e=E)[:, :, 0:D]
                nc.vector.tensor_copy(
                    dst, pj[:, dvh, :].rearrange("p (h e) -> p h e", e=D))

        # ---------- mean over sequence of V_aug ----------
        # ---------- projection groups (q/k for one pair) ----------
        def proj_ops(t):
            """Returns (ops, results): ops is a list of closures, each emits
            one instruction for the q/k projections of pair t."""
            qT_t = qk_pool.tile([P, N], bf16, tag="qT", name=f"qT{t}")
            kT_t = qk_pool.tile([P, N], bf16, tag="kT", name=f"kT{t}")
            ops = []
            state = {}

            def mk_alloc(which):
                def _op():
                    state[which] = ps_pj.tile([P, 2, 512], f32, tag="pj",
                                              name=f"pj{which}")
                return _op

            def mk_mm(which, w_sb, half, kc):
                def _op():
                    nc.tensor.matmul(state[which][:, half, :],
                                     w_sb[:, kc, ts(t, P)],
                                     xT[:, kc, ts(half, 512)],
                                     start=(kc == 0), stop=(kc == KT - 1))
                return _op

            def mk_tt(which, dstT, half):
                def _op():
                    nc.vector.tensor_add(dstT[:, ts(half, 512)],
                                         state[which][:, half, :],
                                         posT[:, t, ts(half, 512)])
                return _op

            for which, w_sb, dstT in (("q", wqT, qT_t), ("k", wkT, kT_t)):
                ops.append(mk_alloc(which))
                # kc-major: both halves reuse the same stationary weights
                for kc in range(KT):
                    for half in range(2):
                        ops.append(mk_mm(which, w_sb, half, kc))
                for half in range(2):
                    ops.append(mk_tt(which, dstT, half))
            return ops, (qT_t, kT_t)

        # pair-0 projections first (only need xT; mean needs V_sb copies)
        ops0, qk0 = proj_ops(0)
        for op in ops0:
            op()

        # head-aligned chunks (7h, 7h, 2h); psum->sbuf copies write mean_sb
        # in (e, h) order so one flat DMA yields mean_cols[e, h].
        mt = ps_pj.tile([P, 2, 512], f32, tag="pj")
        mt2 = ps_st.tile([P, 2, 512], f32, tag="st")
        chunks = ((0, 7, mt[0:1, 0, :]), (7, 7, mt[0:1, 1, :]),
                  (14, 2, mt2[0:1, 0, :]))
        for h0, hn, dstp in chunks:
            for nt in range(NT):
                nc.tensor.matmul(dstp[:, 0:hn * E], ones_col,
                                 V_sb[nt][:, h0 * E:(h0 + hn) * E],
                                 start=(nt == 0), stop=(nt == NT - 1))
        mean_eh = mean_sb.rearrange("o (e h) -> o e h", h=H)  # [1, 64, 16]
        for h0, hn, dstp in chunks:
            nc.vector.tensor_copy(
                mean_eh[:, :, h0:h0 + hn].rearrange("o e h -> o h e"),
                dstp[:, 0:hn * E].rearrange("o (h e) -> o h e", e=E)[:, :, 0:D])
        nc.sync.dma_start(out=mean_cols, in_=mean_sb[0:1, 0:D * H])

        # out-projection group 0, kc 0-6: fed as pair-7 sweep-1 fillers
        oproj_state = {}

        def oproj0_partial_ops():
            ops = []

            def alloc():
                oproj_state["pj"] = ps_pj.tile([P, 2, 512], f32, tag="pj",
                                               name="opj0")
            ops.append(alloc)

            def mk(kc, doh):
                def _op():
                    nc.tensor.matmul(oproj_state["pj"][:, doh, :],
                                     otfull[kc][:, ts(0, P)],
                                     woT[:, kc, ts(doh, 512)],
                                     start=(kc == 0), stop=False)
                return _op

            for kc in range(KT - 1):
                for doh in range(2):
                    ops.append(mk(kc, doh))
            return ops

        # ---------- per-pair attention ----------
        cur_qk = qk0
        for t in range(NPAIR):
            qT_t, kT_t = cur_qk
            if t + 1 < NPAIR:
                pend, cur_qk = proj_ops(t + 1)
            else:
                pend, cur_qk = oproj0_partial_ops(), None
            pend = list(pend)

            for ih in range(2):
                oa = ps_oa.tile([E, 2, 512], f32, tag="oa", name=f"oa{t}_{ih}")
                exs = [None] * NT

                def emit_pv(jt):
                    for hs in range(2):
                        h = 2 * t + hs
                        nc.tensor.matmul(oa[:, hs, :],
                                         V_sb[jt][:, h * E:(h + 1) * E],
                                         exs[jt][:, hs, :],
                                         start=(jt == 0), stop=(jt == NT - 1))

                for jt in range(NT):
                    st = ps_st.tile([P, 2, 512], f32, tag="st")
                    for hs in range(2):
                        nc.tensor.matmul(st[:, hs, :],
                                         kT_t[ts(hs, D), ts(jt, P)],
                                         qT_t[ts(hs, D), ts(ih, 512)],
                                         start=True, stop=True)
                    ex = expool.tile([P, 2, 512], bf16, tag="ex")
                    nc.scalar.activation(ex, st, AF.Exp,
                                         bias=biasj[:, jt:jt + 1], scale=SCALE)
                    exs[jt] = ex
                    # PV lags one jt behind so the exp has a full unit of
                    # slack before the PE needs its output.
                    if jt > 0:
                        emit_pv(jt - 1)
                    # front-loaded so the proj TTs land well before the
                    # next pair's score matmuls need qT/kT. Pair 7's
                    # fillers (out-proj kc<7) wait on pair-6 tails, so
                    # only feed them in sweep ih=1.
                    if t < NPAIR - 1 or ih == 1:
                        for _ in range(3):
                            if pend:
                                pend.pop(0)()
                emit_pv(NT - 1)

                # ---- softmax tail for (t, ih), both heads ----
                oaS = tailp.tile([E, 2, 512], f32, tag="oaS")
                nc.vector.tensor_copy(oaS, oa)     # frees the PSUM acc
                # collect s rows into [p, c] layout: s_coll[p, hs, c] =
                # s_hs[p*4 + c] (both APs flatten row-major -> streams match)
                s_coll = tailp.tile([P, 2, 4], f32, tag="s_coll")
                for hs in range(2):
                    nc.sync.dma_start(out=s_coll[:, hs, :],
                                      in_=oaS[D:D + 1, hs, :])
                r_coll = tailp.tile([P, 2, 4], f32, tag="r_coll")
                nc.vector.reciprocal(r_coll, s_coll)
                nc.vector.tensor_mul(
                    r_coll, r_coll,
                    mcoll[:, ih * 8:(ih + 1) * 8].rearrange(
                        "p (h c) -> p h c", c=4))
                for hs in range(2):
                    h = 2 * t + hs
                    rm_row = tailp.tile([1, 512], f32, tag=f"rm{hs}")
                    nc.sync.dma_start(
                        out=rm_row.rearrange("o (p c) -> o p c", c=4),
                        in_=r_coll[:, hs, :],
                    )
                    rmb = tailp.tile([D, 512], f32, tag=f"rmb{hs}")
                    nc.gpsimd.partition_broadcast(rmb, rm_row)
                    t1 = tailp.tile([D, 512], f32, tag=f"t1{hs}")
                    nc.vector.tensor_mul(t1, oaS[0:D, hs, :], rmb)
                    if hs == 0:
                        nc.vector.scalar_tensor_tensor(
                            otfull[t][0:D, ts(ih, 512)],
                            ommb[:, ts(ih, 512)], mean_cols[:, h:h + 1], t1,
                            OP.mult, OP.add)
                    else:
                        hscr = tailp.tile([D, 512], bf16, tag="hscr")
                        nc.vector.scalar_tensor_tensor(
                            hscr, ommb[:, ts(ih, 512)],
                            mean_cols[:, h:h + 1], t1, OP.mult, OP.add)
                        nc.sync.dma_start(
                            out=otfull[t][D:P, ts(ih, 512)], in_=hscr)
            # any leftover projection ops
            for op in pend:
                op()

        # ---------- out projection ----------
        for nt in range(NT):
            if nt == 0:
                # finish the group started as pair-7 fillers
                pj = oproj_state["pj"]
                for doh in range(2):
                    nc.tensor.matmul(pj[:, doh, :],
                                     otfull[KT - 1][:, ts(0, P)],
                                     woT[:, KT - 1, ts(doh, 512)],
                                     start=False, stop=True)
            else:
                pool, tg = (ps_st, "st") if nt % 2 else (ps_pj, "pj")
                pj = pool.tile([P, 2, 512], f32, tag=tg)
                for kc in range(KT):
                    for doh in range(2):
                        nc.tensor.matmul(pj[:, doh, :],
                                         otfull[kc][:, ts(nt, P)],
                                         woT[:, kc, ts(doh, 512)],
                                         start=(kc == 0), stop=(kc == KT - 1))
            for doh in range(2):
                ostage = tailp.tile([P, 512], f32, tag="ostage")
                nc.vector.tensor_add(ostage, pj[:, doh, :],
                                     b_bcast[:, ts(doh, 512)])
                eng = nc.sync if (nt + doh) % 2 == 0 else nc.scalar
                eng.dma_start(out=out_d[ts(nt, P), ts(doh, 512)],
                              in_=ostage)

    nc.finalize()
    return nc


def _host_prep(x, mask, pos, W_qk, W_v, W_out, b_out):
    bf = ml_dtypes.bfloat16
    x = np.ascontiguousarray(x, dtype=np.float32)
    pos = np.ascontiguousarray(pos, dtype=np.float32)
    W_qk = np.asarray(W_qk, dtype=np.float32)
    maskf = np.concatenate(
        [np.ones((B, 1), np.float32), np.asarray(mask).astype(np.float32)],
        axis=1)                                        # [B, N]
    wqT = np.ascontiguousarray(W_qk[:DIM].T.astype(bf))
    wkT = np.ascontiguousarray(W_qk[DIM:].T.astype(bf))
    wvT = np.ascontiguousarray(np.asarray(W_v, np.float32).T.astype(bf))
    woT = np.ascontiguousarray(np.asarray(W_out, np.float32).T.astype(bf))
    b_out = np.ascontiguousarray(b_out, dtype=np.float32)

    in_maps = []
    for b in range(B):
        m = maskf[b]
        biasj = np.ascontiguousarray(
            (MB * m - MB).reshape(NT, P).T)            # [p, jt]
        # mcoll[p, ih*8 + hs*4 + c] = m[ih*512 + p*4 + c] (dup for both heads)
        mc = m.reshape(2, P, 4)                        # [ih, p, c]
        mcoll = np.ascontiguousarray(
            np.stack([mc[0], mc[0], mc[1], mc[1]],
                     axis=1).reshape(P, 16))
        in_maps.append({
            "xT": np.ascontiguousarray(x[b].T.astype(bf)),
            "posT": np.ascontiguousarray(pos[b].T.astype(bf)),
            "wqT": wqT, "wkT": wkT, "wvT": wvT, "woT": woT,
            "biasj": biasj.astype(np.float32),
            "mcoll": mcoll.astype(np.float32),
            "omm": np.ascontiguousarray(1.0 - m),
            "b_out": b_out,
        })
    return in_maps


def kernel(x, mask, pos, W_qk, W_v, W_out, b_out):
    global _NC
    from concourse.bass_utils import run_bass_kernel_spmd

    if _NC is None:
        _NC = _build()

    in_maps = _host_prep(x, mask, pos, W_qk, W_v, W_out, b_out)
    res = run_bass_kernel_spmd(_NC, in_maps, core_ids=list(range(B)))
    return np.stack([res.results[b]["out"] for b in range(B)]).astype(np.float32)


# revision 26
# speedup vs baseline: 1.7515x; 1.0507x over previous
"""Multi-head attention kernel for Trainium2, batch-parallel across 8 NeuronCores.

Reference (per batch element b, one core each):
  qk = x @ W_qk.T ; q,k = split(qk) ; v = x @ W_v.T
  q,k,v -> [h, n, d] ; q += pos_h ; k += pos_h
  S = q @ k.T * DIM**-0.5 ; mask = outer(m, m) ; masked -> -inf
  P = softmax(S) ; O = P @ v ; out = merge_heads(O) @ W_out.T + b_out

Device strategy (per core):
  - all layout work (transposes, bf16 casts, mask-derived tensors) done on
    HOST in numpy; the device receives x^T, pos^T, W_q^T, W_k^T, W_v^T,
    W_out^T in bf16 and streams pure matmuls.
  - scores computed TRANSPOSED per head pair: the two heads' K=64 score
    matmuls go to row groups 0-1 / 2-3 of the PE array (base partitions 0
    and 64) and run CONCURRENTLY (row tiling).
  - exp via one 1024-wide ACT per (pair, jt, ih) over both heads' scores in
    a [128, 2, 512] PSUM tile; the column mask folds into the per-partition
    exp bias, softmax row sums come from an appended ones-column in the PV
    matmul (V_aug = [V_h | 1], M=65).
  - per-pair attention is two i-half sweeps (ih=0,1) so both heads' PV
    accumulators fit one 2-bank PSUM tile; the softmax tail (1/s, row mask,
    masked-row blend with mean(V)) runs per (ih, head) off the PE path.
  - the next pair's q/k projections are interleaved into the attention
    units so the PE never waits on the ACT engine.
  - PSUM budget exactly 8 banks: score ring 2x2 + proj 2 + PV acc 2.
"""
import sys

sys.path.insert(0, "/opt/trn_rl_repo")

import numpy as np
import ml_dtypes
from contextlib import ExitStack

B, N, DIM, H = 8, 1024, 1024, 16
D = DIM // H          # 64
E = D + 1             # V_aug block (64 cols of V + ones column)
P = 128
NT = N // P           # 8 n-tiles
KT = DIM // P         # 8 k-tiles
NPAIR = H // 2        # 8 head pairs
SCALE = DIM ** (-0.5)
MB = 30.0             # mask bias magnitude: bias_j = 30*m - 30 in {0, -30}

_NC = None


def _build():
    import concourse.bacc as bacc
    import concourse.bass as bass
    import concourse.mybir as mybir
    import concourse.tile as tile

    f32 = mybir.dt.float32
    bf16 = mybir.dt.bfloat16
    AF = mybir.ActivationFunctionType
    OP = mybir.AluOpType
    ts = bass.ts

    nc = bacc.Bacc()
    xT_d = nc.declare_dram_parameter("xT", [DIM, N], bf16, isOutput=False)
    posT_d = nc.declare_dram_parameter("posT", [DIM, N], bf16, isOutput=False)
    wqT_d = nc.declare_dram_parameter("wqT", [DIM, DIM], bf16, isOutput=False)
    wkT_d = nc.declare_dram_parameter("wkT", [DIM, DIM], bf16, isOutput=False)
    wvT_d = nc.declare_dram_parameter("wvT", [DIM, DIM], bf16, isOutput=False)
    woT_d = nc.declare_dram_parameter("woT", [DIM, DIM], bf16, isOutput=False)
    biasj_d = nc.declare_dram_parameter("biasj", [P, NT], f32, isOutput=False)
    mcoll_d = nc.declare_dram_parameter("mcoll", [P, 16], f32, isOutput=False)
    omm_d = nc.declare_dram_parameter("omm", [N], f32, isOutput=False)
    b_d = nc.declare_dram_parameter("b_out", [DIM], f32, isOutput=False)
    out_d = nc.declare_dram_parameter("out", [N, DIM], f32, isOutput=True)

    with ExitStack() as ctx:
        tc = ctx.enter_context(tile.TileContext(nc))
        sing = ctx.enter_context(tc.tile_pool(name="sing", bufs=1))
        qk_pool = ctx.enter_context(tc.tile_pool(name="qk", bufs=2))
        expool = ctx.enter_context(tc.tile_pool(name="expool", bufs=4))
        tailp = ctx.enter_context(tc.tile_pool(name="tailp", bufs=2))
        ps_st = ctx.enter_context(tc.tile_pool(name="ps_st", bufs=2, space="PSUM"))
        ps_pj = ctx.enter_context(tc.tile_pool(name="ps_pj", bufs=1, space="PSUM"))
        ps_oa = ctx.enter_context(tc.tile_pool(name="ps_oa", bufs=1, space="PSUM"))

        # ---------- persistent SBUF ----------
        xT = sing.tile([P, KT, N], bf16, tag="xT")
        posT = sing.tile([P, KT, N], bf16, tag="posT")
        wqT = sing.tile([P, KT, DIM], bf16, tag="wqT")
        wkT = sing.tile([P, KT, DIM], bf16, tag="wkT")
        wvT = sing.tile([P, KT, DIM], bf16, tag="wvT")
        woT = sing.tile([P, KT, DIM], bf16, tag="woT")
        V_sb = [sing.tile([P, H * E], bf16, tag=f"V{nt}", name=f"V{nt}")
                for nt in range(NT)]
        otfull = [sing.tile([P, N], bf16, tag=f"otf{kc}", name=f"otf{kc}")
                  for kc in range(KT)]
        biasj = sing.tile([P, NT], f32, tag="biasj")
        mcoll = sing.tile([P, 16], f32, tag="mcoll")
        omm_row = sing.tile([1, N], f32, tag="omm_row")
        ommb = sing.tile([D, N], f32, tag="ommb")
        b_row = sing.tile([1, DIM], f32, tag="b_row")
        b_bcast = sing.tile([P, DIM], f32, tag="b_bcast")
        mean_sb = sing.tile([1, D * H], f32, tag="mean_sb")
        mean_cols = sing.tile([D, H], f32, tag="mean_cols")

        # ---------- input DMAs ----------
        # Two parallel hwdge queues (sync + scalar), each ~190 GB/s. Order
        # so V-proj deps (wvT + xT) land first, then pair-0's proj deps.
        # xT loads are per-token-block so V-proj group nt waits only on
        # block nt; wq/wk loads are per-pair slices.
        xTd_v = xT_d.rearrange("(kc p) t -> p kc t", p=P)
        wq_v = wqT_d.rearrange("(kc p) d -> p kc d", p=P)
        wk_v = wkT_d.rearrange("(kc p) d -> p kc d", p=P)
        nc.sync.dma_start(out=xT[:, :, ts(0, P)], in_=xTd_v[:, :, ts(0, P)])
        nc.scalar.dma_start(out=xT[:, :, ts(1, P)], in_=xTd_v[:, :, ts(1, P)])
        for kc in range(0, KT, 2):
            nc.sync.dma_start(out=wvT[:, kc, :], in_=wvT_d[ts(kc, P), :])
        for kc in range(1, KT, 2):
            nc.scalar.dma_start(out=wvT[:, kc, :], in_=wvT_d[ts(kc, P), :])
        for nt in range(2, NT, 2):
            nc.sync.dma_start(out=xT[:, :, ts(nt, P)], in_=xTd_v[:, :, ts(nt, P)])
        for nt in range(3, NT, 2):
            nc.scalar.dma_start(out=xT[:, :, ts(nt, P)], in_=xTd_v[:, :, ts(nt, P)])
        nc.scalar.dma_start(out=biasj, in_=biasj_d[:, :])
        nc.scalar.dma_start(out=mcoll, in_=mcoll_d[:, :])
        nc.scalar.dma_start(out=omm_row, in_=omm_d[:])
        nc.scalar.dma_start(out=b_row, in_=b_d[:])
        for t in range(NPAIR):
            nc.sync.dma_start(out=wqT[:, :, ts(t, P)], in_=wq_v[:, :, ts(t, P)])
            nc.scalar.dma_start(out=wkT[:, :, ts(t, P)], in_=wk_v[:, :, ts(t, P)])
            nc.scalar.dma_start(out=posT[:, t, :], in_=posT_d[ts(t, P), :])
        for kc in range(0, KT, 2):
            nc.sync.dma_start(out=woT[:, kc, :], in_=woT_d[ts(kc, P), :])
        for kc in range(1, KT, 2):
            nc.scalar.dma_start(out=woT[:, kc, :], in_=woT_d[ts(kc, P), :])

        # ---------- small prep (off-PE) ----------
        nc.gpsimd.partition_broadcast(ommb, omm_row)
        nc.gpsimd.partition_broadcast(b_bcast, b_row)
        const1 = sing.tile([P, H], f32, tag="const1")
        nc.vector.memset(const1, 1.0)
        for nt in range(NT):
            ones_ap = V_sb[nt].rearrange("p (h e) -> p h e", e=E)[:, :, D:E]
            nc.vector.tensor_copy(ones_ap.squeeze(), const1)
        constN = sing.tile([P, 1], f32, tag="constN")
        nc.vector.memset(constN, 1.0 / N)
        ones_col = sing.tile([P, 1], bf16, tag="ones_col")
        nc.vector.tensor_copy(ones_col, constN)

        # ---------- V = x @ W_v.T  (stored as [V_h | 1] x 16 heads) ----------
        for nt in range(NT):
            pool, tg = (ps_st, "st") if nt % 2 else (ps_pj, "pj")
            pj = pool.tile([P, 2, 512], f32, tag=tg)
            # kc-major so consecutive matmuls share the stationary operand
            for kc in range(KT):
                for dvh in range(2):
                    nc.tensor.matmul(pj[:, dvh, :], xT[:, kc, ts(nt, P)],
                                     wvT[:, kc, ts(dvh, 512)],
                                     start=(kc == 0), stop=(kc == KT - 1))
            for dvh in range(2):
                dst = V_sb[nt][:, dvh * 8 * E: dvh * 8 * E + 8 * E].rearrange(
                    "p (h e) -> p h e", e=E)[:, :, 0:D]
                nc.vector.tensor_copy(
                    dst, pj[:, dvh, :].rearrange("p (h e) -> p h e", e=D))

        # ---------- mean over sequence of V_aug ----------
        # ---------- projection groups (q/k for one pair) ----------
        def proj_ops(t):
            """Returns (ops, results): ops is a list of closures, each emits
            one instruction for the q/k projections of pair t."""
            qT_t = qk_pool.tile([P, N], bf16, tag="qT", name=f"qT{t}")
            kT_t = qk_pool.tile([P, N], bf16, tag="kT", name=f"kT{t}")
            ops = []
            state = {}

            def mk_alloc(which):
                def _op():
                    state[which] = ps_pj.tile([P, 2, 512], f32, tag="pj",
                                              name=f"pj{which}")
                return _op

            def mk_mm(which, w_sb, half, kc):
                def _op():
                    nc.tensor.matmul(state[which][:, half, :],
                                     w_sb[:, kc, ts(t, P)],
                                     xT[:, kc, ts(half, 512)],
                                     start=(kc == 0), stop=(kc == KT - 1))
                return _op

            def mk_tt(which, dstT, half):
                def _op():
                    nc.vector.tensor_add(dstT[:, ts(half, 512)],
                                         state[which][:, half, :],
                                         posT[:, t, ts(half, 512)])
                return _op

            for which, w_sb, dstT in (("q", wqT, qT_t), ("k", wkT, kT_t)):
                ops.append(mk_alloc(which))
                # kc-major: both halves reuse the same stationary weights
                for kc in range(KT):
                    for half in range(2):
                        ops.append(mk_mm(which, w_sb, half, kc))
                for half in range(2):
                    ops.append(mk_tt(which, dstT, half))
            return ops, (qT_t, kT_t)

        # pair-0 projections first (only need xT; mean needs V_sb copies)
        ops0, qk0 = proj_ops(0)
        for op in ops0:
            op()

        # head-aligned chunks (7h, 7h, 2h); psum->sbuf copies write mean_sb
        # in (e, h) order so one flat DMA yields mean_cols[e, h].
        mt = ps_pj.tile([P, 2, 512], f32, tag="pj")
        mt2 = ps_st.tile([P, 2, 512], f32, tag="st")
        chunks = ((0, 7, mt[0:1, 0, :]), (7, 7, mt[0:1, 1, :]),
                  (14, 2, mt2[0:1, 0, :]))
        for h0, hn, dstp in chunks:
            for nt in range(NT):
                nc.tensor.matmul(dstp[:, 0:hn * E], ones_col,
                                 V_sb[nt][:, h0 * E:(h0 + hn) * E],
                                 start=(nt == 0), stop=(nt == NT - 1))
        mean_eh = mean_sb.rearrange("o (e h) -> o e h", h=H)  # [1, 64, 16]
        for h0, hn, dstp in chunks:
            nc.vector.tensor_copy(
                mean_eh[:, :, h0:h0 + hn].rearrange("o e h -> o h e"),
                dstp[:, 0:hn * E].rearrange("o (h e) -> o h e", e=E)[:, :, 0:D])
        nc.sync.dma_start(out=mean_cols, in_=mean_sb[0:1, 0:D * H])

        # out-projection group 0, kc 0-6: fed as pair-7 sweep-1 fillers
        oproj_state = {}

        def oproj0_partial_ops():
            ops = []

            def alloc():
                oproj_state["pj"] = ps_pj.tile([P, 2, 512], f32, tag="pj",
                                               name="opj0")
            ops.append(alloc)

            def mk(kc, doh):
                def _op():
                    nc.tensor.matmul(oproj_state["pj"][:, doh, :],
                                     otfull[kc][:, ts(0, P)],
                                     woT[:, kc, ts(doh, 512)],
                                     start=(kc == 0), stop=False)
                return _op

            for kc in range(KT - 1):
                for doh in range(2):
                    ops.append(mk(kc, doh))
            return ops

        # ---------- per-pair attention ----------
        cur_qk = qk0
        for t in range(NPAIR):
            qT_t, kT_t = cur_qk
            if t + 1 < NPAIR:
                pend, cur_qk = proj_ops(t + 1)
            else:
                pend, cur_qk = oproj0_partial_ops(), None
            pend = list(pend)

            for ih in range(2):
                oaf = ps_oa.tile([P, 2, 512], f32, tag="oa", name=f"oa{t}_{ih}")
                oa = oaf[0:E, :, :]
                exs = [None] * NT

                def emit_pv(jt):
                    for hs in range(2):
                        h = 2 * t + hs
                        nc.tensor.matmul(oa[:, hs, :],
                                         V_sb[jt][:, h * E:(h + 1) * E],
                                         exs[jt][:, hs, :],
                                         start=(jt == 0), stop=(jt == NT - 1))

                for jt in range(NT):
                    st = ps_st.tile([P, 2, 512], f32, tag="st")
                    for hs in range(2):
                        nc.tensor.matmul(st[:, hs, :],
                                         kT_t[ts(hs, D), ts(jt, P)],
                                         qT_t[ts(hs, D), ts(ih, 512)],
                                         start=True, stop=True)
                    ex = expool.tile([P, 2, 512], bf16, tag="ex")
                    nc.scalar.activation(ex, st, AF.Exp,
                                         bias=biasj[:, jt:jt + 1], scale=SCALE)
                    exs[jt] = ex
                    # PV lags one jt behind (two at sweep start, giving the
                    # oaS drain of the previous sweep room to free the acc).
                    if jt == 1:
                        pass
                    elif jt > 1:
                        emit_pv(jt - 2)
                    # front-loaded so the proj TTs land well before the
                    # next pair's score matmuls need qT/kT. Pair 7's
                    # fillers (out-proj kc<7) wait on pair-6 tails, so
                    # only feed them in sweep ih=1.
                    if t < NPAIR - 1 or ih == 1:
                        for _ in range(3):
                            if pend:
                                pend.pop(0)()
                emit_pv(NT - 2)
                emit_pv(NT - 1)

                # ---- softmax tail for (t, ih), both heads ----
                oaS = tailp.tile([E, 2, 512], f32, tag="oaS")
                nc.vector.tensor_copy(oaS, oa)     # frees the PSUM acc
                # collect s rows into [p, c] layout: s_coll[p, hs, c] =
                # s_hs[p*4 + c] (both APs flatten row-major -> streams match)
                s_coll = tailp.tile([P, 2, 4], f32, tag="s_coll")
                for hs in range(2):
                    nc.sync.dma_start(out=s_coll[:, hs, :],
                                      in_=oaS[D:D + 1, hs, :])
                r_coll = tailp.tile([P, 2, 4], f32, tag="r_coll")
                nc.vector.reciprocal(r_coll, s_coll)
                nc.vector.tensor_mul(
                    r_coll, r_coll,
                    mcoll[:, ih * 8:(ih + 1) * 8].rearrange(
                        "p (h c) -> p h c", c=4))
                for hs in range(2):
                    h = 2 * t + hs
                    rm_row = tailp.tile([1, 512], f32, tag=f"rm{hs}")
                    nc.sync.dma_start(
                        out=rm_row.rearrange("o (p c) -> o p c", c=4),
                        in_=r_coll[:, hs, :],
                    )
                    rmb = tailp.tile([D, 512], f32, tag=f"rmb{hs}")
                    nc.gpsimd.partition_broadcast(rmb, rm_row)
                    t1 = tailp.tile([D, 512], f32, tag=f"t1{hs}")
                    nc.vector.tensor_mul(t1, oaS[0:D, hs, :], rmb)
                    if hs == 0:
                        nc.vector.scalar_tensor_tensor(
                            otfull[t][0:D, ts(ih, 512)],
                            ommb[:, ts(ih, 512)], mean_cols[:, h:h + 1], t1,
                            OP.mult, OP.add)
                    else:
                        hscr = tailp.tile([D, 512], bf16, tag="hscr")
                        nc.vector.scalar_tensor_tensor(
                            hscr, ommb[:, ts(ih, 512)],
                            mean_cols[:, h:h + 1], t1, OP.mult, OP.add)
                        nc.sync.dma_start(
                            out=otfull[t][D:P, ts(ih, 512)], in_=hscr)
            # any leftover projection ops
            for op in pend:
                op()

        # ---------- out projection (3 psum rings: st, oa, pj) ----------
        for nt in range(NT):
            if nt == 0:
                # finish the group started as pair-7 fillers
                pj = oproj_state["pj"]
                for doh in range(2):
                    nc.tensor.matmul(pj[:, doh, :],
                                     otfull[KT - 1][:, ts(0, P)],
                                     woT[:, KT - 1, ts(doh, 512)],
                                     start=False, stop=True)
            else:
                pool, tg = ((ps_st, "st"), (ps_oa, "oa"),
                            (ps_pj, "pj"))[nt % 3]
                pj = pool.tile([P, 2, 512], f32, tag=tg)
                for kc in range(KT):
                    for doh in range(2):
                        nc.tensor.matmul(pj[:, doh, :],
                                         otfull[kc][:, ts(nt, P)],
                                         woT[:, kc, ts(doh, 512)],
                                         start=(kc == 0), stop=(kc == KT - 1))
            for doh in range(2):
                ostage = tailp.tile([P, 512], f32, tag="ostage")
                nc.vector.tensor_add(ostage, pj[:, doh, :],
                                     b_bcast[:, ts(doh, 512)])
                eng = nc.sync if (nt + doh) % 2 == 0 else nc.scalar
                eng.dma_start(out=out_d[ts(nt, P), ts(doh, 512)],
                              in_=ostage)

    nc.finalize()
    return nc


def _host_prep(x, mask, pos, W_qk, W_v, W_out, b_out):
    bf = ml_dtypes.bfloat16
    x = np.ascontiguousarray(x, dtype=np.float32)
    pos = np.ascontiguousarray(pos, dtype=np.float32)
    W_qk = np.asarray(W_qk, dtype=np.float32)
    maskf = np.concatenate(
        [np.ones((B, 1), np.float32), np.asarray(mask).astype(np.float32)],
        axis=1)                                        # [B, N]
    wqT = np.ascontiguousarray(W_qk[:DIM].T.astype(bf))
    wkT = np.ascontiguousarray(W_qk[DIM:].T.astype(bf))
    wvT = np.ascontiguousarray(np.asarray(W_v, np.float32).T.astype(bf))
    woT = np.ascontiguousarray(np.asarray(W_out, np.float32).T.astype(bf))
    b_out = np.ascontiguousarray(b_out, dtype=np.float32)

    in_maps = []
    for b in range(B):
        m = maskf[b]
        biasj = np.ascontiguousarray(
            (MB * m - MB).reshape(NT, P).T)            # [p, jt]
        # mcoll[p, ih*8 + hs*4 + c] = m[ih*512 + p*4 + c] (dup for both heads)
        mc = m.reshape(2, P, 4)                        # [ih, p, c]
        mcoll = np.ascontiguousarray(
            np.stack([mc[0], mc[0], mc[1], mc[1]],
                     axis=1).reshape(P, 16))
        in_maps.append({
            "xT": np.ascontiguousarray(x[b].T.astype(bf)),
            "posT": np.ascontiguousarray(pos[b].T.astype(bf)),
            "wqT": wqT, "wkT": wkT, "wvT": wvT, "woT": woT,
            "biasj": biasj.astype(np.float32),
            "mcoll": mcoll.astype(np.float32),
            "omm": np.ascontiguousarray(1.0 - m),
            "b_out": b_out,
        })
    return in_maps


def kernel(x, mask, pos, W_qk, W_v, W_out, b_out):
    global _NC
    from concourse.bass_utils import run_bass_kernel_spmd

    if _NC is None:
        _NC = _build()

    in_maps = _host_prep(x, mask, pos, W_qk, W_v, W_out, b_out)
    res = run_bass_kernel_spmd(_NC, in_maps, core_ids=list(range(B)))
    return np.stack([res.results[b]["out"] for b in range(B)]).astype(np.float32)
